# revision 16
# baseline (speedup 1.0000x reference)
"""Trainium2 Bass kernel for a 16-head decoder self-attention block.

Reference computation (B=2, S=2048, E=2048, H=16, D=128):
    qkv = X @ W_qkv.T + b_qkv ; RoPE(Q, K) ; attn = softmax(QK^T/sqrt(D) + mask)
    out = (attn @ V reshaped) @ W_o.T + b_o

Sharding over 8 NeuronCores: data parallel over batch (2) x tensor parallel
over 4 head-groups of 4 heads each. Each core computes its group's qkv
projection, attention, and a partial (rank-512) slice of the output
projection; the host sums the 4 partials per batch element.

v4: all matmul operands bf16 (PSUM stays fp32), everything SBUF-resident.
The attention phase is ScalarE(exp)-bound with only ~80% PE duty, which
makes the PE's DVFS drop it to a lower p-state; to keep the PE saturated
(and at max clock) the kernel interleaves deferred independent matmul work
into the attention stream:
  - q-chunk 0 gets the deferred Q-projection of sequence quarters 2-3
    (only needed by q-chunk 1);
  - q-chunk 1 gets q-chunk 0's output projection (phase 3), whose PSUM
    eviction runs on DVE (tensor_scalar bias add) to keep ScalarE free;
  - the softmax denominator is accumulated on DVE in bf16 (plus one
    ones-matmul per chunk), not on the PE.
V is projected X-tile-stationary directly in [s, d] layout (no PE
transposes); Q/K evictions fuse bias (ACT) + RoPE (perm matmul + DVE).
"""

import math
import sys

import numpy as np

sys.path.insert(0, "/opt/trn_rl_repo")

B, S, E = 2, 2048, 2048
H, D = 16, 128
NCORES = 8
NGROUP = 4          # head groups (tensor parallel)
HPG = H // NGROUP   # heads per group = 4
GE = HPG * D        # group embed width = 512
KT = E // 128       # contraction tiles over E = 16
ST = S // 128       # sequence tiles = 16
MQK = 2 * HPG       # q+k m-tiles per core = 8
SCALE = 1.0 / math.sqrt(D)
NQC = 2             # q-chunks of 1024
QW = S // NQC

_CACHE = {}


def _build():
    """Build + compile the per-core Bass program (same program, all cores)."""
    import concourse.bacc as bacc
    import concourse.mybir as mybir
    import concourse.tile as tile

    F32 = mybir.dt.float32
    BF16 = mybir.dt.bfloat16
    EXP = mybir.ActivationFunctionType.Exp
    IDENT = mybir.ActivationFunctionType.Identity

    nc = bacc.Bacc("TRN2", target_bir_lowering=False, debug=False)

    xt = nc.dram_tensor("xt", [E, S], BF16, kind="ExternalInput").ap()       # X[b].T
    w1qk = nc.dram_tensor("w1qk", [MQK * 128, E], BF16, kind="ExternalInput").ap()
    w1v = nc.dram_tensor("w1v", [E, GE], BF16, kind="ExternalInput").ap()    # Wv_g^T
    w2t = nc.dram_tensor("w2t", [KT * 128, GE], BF16, kind="ExternalInput").ap()
    bqkv = nc.dram_tensor("bqkv", [128, MQK], F32, kind="ExternalInput").ap()
    bvb = nc.dram_tensor("bvb", [128, GE], F32, kind="ExternalInput").ap()   # b_v bcast
    bo = nc.dram_tensor("bo", [128, KT], F32, kind="ExternalInput").ap()
    mb = nc.dram_tensor("mb", [128, ST], F32, kind="ExternalInput").ap()     # mask bias
    cosx = nc.dram_tensor("cosx", [128, S], BF16, kind="ExternalInput").ap()
    sinx = nc.dram_tensor("sinx", [128, S], BF16, kind="ExternalInput").ap()
    ones = nc.dram_tensor("ones", [128, 128], BF16, kind="ExternalInput").ap()
    perm = nc.dram_tensor("perm", [128, 128], BF16, kind="ExternalInput").ap()
    pout = nc.dram_tensor("pout", [E, S], BF16, kind="ExternalOutput").ap()

    QC = 512
    NCH = S // QC            # 4 s-quarters
    SPC = QC // 128          # s-tiles per quarter = 4
    NDEF = 2                 # deferred Q quarters (2, 3)

    with tile.TileContext(nc) as tc:
        with tc.tile_pool(name="small", bufs=1) as spool, \
             tc.tile_pool(name="qkout", bufs=1) as qkpool, \
             tc.tile_pool(name="vout", bufs=1) as vpool, \
             tc.tile_pool(name="at", bufs=1) as atpool, \
             tc.tile_pool(name="xq", bufs=2) as xqp:
            # long-lived SBUF tensors
            qk = [qkpool.tile([128, S], BF16, tag=f"qk{m}", name=f"qk{m}")
                  for m in range(MQK)]                       # rope'd Q (0-3), K (4-7)
            v_sb = [vpool.tile([128, GE], BF16, tag=f"v{st_}", name=f"v{st_}")
                    for st_ in range(ST)]                    # V tiles [sk, d-group]
            at_tiles = [atpool.tile([128, S], BF16, tag=f"at{h}", name=f"at{h}")
                        for h in range(HPG)]

            # PE warm-up: dummy matmuls during the initial DMA window keep
            # the tensor engine's DVFS ramped before the first real chain
            with tc.tile_pool(name="warm", bufs=1) as warmp, \
                 tc.tile_pool(name="wps", bufs=2, space="PSUM") as wpsp:
                wt_ = warmp.tile([128, QC], BF16, tag="warm")
                nc.vector.memset(wt_[:], 0.0)
                for i in range(22):
                    wps = wpsp.tile([128, QC], F32, tag="wps")
                    nc.tensor.matmul(wps[:], wt_[:, 0:128], wt_[:],
                                     start=True, stop=True)

            # startup DMAs, in need order: first V chain needs xq0[k]+w1v[k]
            xq_sets = [None] * NCH
            xq_sets[0] = []
            w1v_sb = []
            for k in range(KT):
                t = xqp.tile([128, QC], BF16, tag=f"xq{k}", name=f"xq0_{k}")
                nc.sync.dma_start(t[:], xt[k * 128:(k + 1) * 128, 0:QC])
                xq_sets[0].append(t)
                w = spool.tile([128, GE], BF16, tag=f"w1v{k}", name=f"w1v{k}")
                nc.sync.dma_start(w[:], w1v[k * 128:(k + 1) * 128, :])
                w1v_sb.append(w)
            bvb_sb = spool.tile([128, GE], F32, tag="bvb")
            nc.sync.dma_start(bvb_sb[:], bvb)
            w1m = [spool.tile([128, E], BF16, tag=f"w1m{m}", name=f"w1m{m}")
                   for m in range(MQK)]
            nc.sync.dma_start(w1m[0][:], w1qk[0:128, :])
            bq_sb = spool.tile([128, MQK], F32, tag="bq")
            nc.sync.dma_start(bq_sb[:], bqkv)
            perm_sb = spool.tile([128, 128], BF16, tag="perm")
            nc.sync.dma_start(perm_sb[:], perm)
            cos_sb = spool.tile([128, S], BF16, tag="cos")
            nc.sync.dma_start(cos_sb[:], cosx)
            sin_sb = spool.tile([128, S], BF16, tag="sin")
            nc.sync.dma_start(sin_sb[:], sinx)
            for m in range(1, MQK):
                nc.sync.dma_start(w1m[m][:], w1qk[m * 128:(m + 1) * 128, :])
            ones_sb = spool.tile([128, 128], BF16, tag="ones")
            nc.sync.dma_start(ones_sb[:], ones)
            mb_sb = spool.tile([128, ST], F32, tag="mb")
            nc.sync.dma_start(mb_sb[:], mb)
            bo_sb = spool.tile([128, KT], F32, tag="bo")
            nc.sync.dma_start(bo_sb[:], bo)

            with tc.tile_pool(name="qbp", bufs=3) as qbp, \
                 tc.tile_pool(name="rap", bufs=2) as rap, \
                 tc.tile_pool(name="stp", bufs=2) as stp:

                def qk_proj(m, ch, ps_pool):
                    """Q/K m-tile projection matmuls + ACT bias evict -> qb."""
                    xq = xq_sets[ch]
                    ps_t = ps_pool.tile([128, ps_pool._qk_width], F32,
                                        tag=ps_pool._qk_tag,
                                        name=f"qps{m}_{ch}")
                    ps = ps_t[:, 0:QC]
                    for k in range(KT):
                        nc.tensor.matmul(
                            ps, w1m[m][:, k * 128:(k + 1) * 128],
                            xq[k][:], start=(k == 0), stop=(k == KT - 1))
                    qb = qbp.tile([128, QC], BF16, tag="qb")
                    nc.scalar.activation(qb[:], ps, IDENT,
                                         bias=bq_sb[:, m:m + 1], scale=1.0)
                    return qb

                def qk_rope(qb, m, ch, ps2_pool):
                    """RoPE: trans() is a signed dim-permutation matmul.
                    Emitted one unit behind the projection so the perm
                    matmul never waits on the ACT eviction."""
                    cs = slice(ch * QC, (ch + 1) * QC)
                    ps2_t = ps2_pool.tile([128, ps2_pool._qk_width], F32,
                                          tag=ps2_pool._qk_tag,
                                          name=f"qps2{m}_{ch}")
                    ps2 = ps2_t[:, 0:QC]
                    nc.tensor.matmul(ps2, perm_sb[:], qb[:],
                                     start=True, stop=True)
                    ra = rap.tile([128, QC], BF16, tag="ra")
                    nc.vector.tensor_mul(ra[:], qb[:], cos_sb[:, cs])
                    st = stp.tile([128, QC], BF16, tag="st")
                    nc.vector.tensor_mul(st[:], ps2, sin_sb[:, cs])
                    nc.vector.tensor_add(qk[m][:, cs], st[:], ra[:])

                # ---------------- Phase 1 (s-quarters; Q of quarters 2-3 deferred)
                with tc.tile_pool(name="vps", bufs=2, space="PSUM") as vps_pool, \
                     tc.tile_pool(name="qkps", bufs=2, space="PSUM") as qkps, \
                     tc.tile_pool(name="qkps2", bufs=2, space="PSUM") as qkps2:
                    qkps._qk_tag = "ps"
                    qkps._qk_width = QC
                    qkps2._qk_tag = "ps2"
                    qkps2._qk_width = QC
                    pend = [None]
                    for ch in range(NCH):
                        if ch + 1 < NCH:
                            ncs = slice((ch + 1) * QC, (ch + 2) * QC)
                            xq_sets[ch + 1] = []
                            for k in range(KT):
                                t = xqp.tile([128, QC], BF16, tag=f"xq{k}",
                                             name=f"xq{ch + 1}_{k}")
                                nc.sync.dma_start(
                                    t[:], xt[k * 128:(k + 1) * 128, ncs])
                                xq_sets[ch + 1].append(t)
                        # V projection: X-tile stationary, [s, d] output layout
                        for sl_ in range(SPC):
                            st_ = ch * SPC + sl_
                            ps = vps_pool.tile([128, GE], F32, tag="vps")
                            for k in range(KT):
                                nc.tensor.matmul(
                                    ps[:], xq_sets[ch][k][:, sl_ * 128:(sl_ + 1) * 128],
                                    w1v_sb[k][:],
                                    start=(k == 0), stop=(k == KT - 1))
                            nc.vector.tensor_add(v_sb[st_][:], ps[:], bvb_sb[:])
                        # K always; Q only for quarters 0-1 (rest deferred)
                        for m in range(MQK):
                            if m < HPG and ch >= NCH - NDEF:
                                continue
                            qb = qk_proj(m, ch, qkps)
                            if pend[0] is not None:
                                qk_rope(*pend[0], qkps2)
                            pend[0] = (qb, m, ch)
                    if pend[0] is not None:
                        qk_rope(*pend[0], qkps2)

                # W2 resident; DMAs land during early phase 2
                w2s = []
                for m in range(KT):
                    t = spool.tile([128, GE], BF16, tag=f"w2{m}", name=f"w2{m}")
                    nc.sync.dma_start(t[:], w2t[m * 128:(m + 1) * 128, :])
                    w2s.append(t)

                # ---------------- Phase 2 (+ deferred Q, + interleaved phase 3)
                with tc.tile_pool(name="ex", bufs=5) as exp_pool, \
                     tc.tile_pool(name="dac", bufs=2) as dac_pool, \
                     tc.tile_pool(name="rc", bufs=1) as rcp, \
                     tc.tile_pool(name="osb", bufs=2) as osbp, \
                     tc.tile_pool(name="ost", bufs=2) as ost, \
                     tc.tile_pool(name="pss", bufs=2, space="PSUM") as pss_pool, \
                     tc.tile_pool(name="pso", bufs=1, space="PSUM") as pso_pool, \
                     tc.tile_pool(name="ops", bufs=1, space="PSUM") as ops_pool:
                    pss_pool._qk_tag = "pss"
                    pss_pool._qk_width = QW

                    def p3_unit(m, qc, pool):
                        """One output-projection m-tile for one q-chunk."""
                        ps = pool.tile([128, QW], F32, tag="ops")
                        for k in range(HPG):
                            for ns in range(2):
                                sl_o = slice(ns * 512, (ns + 1) * 512)
                                sl_i = slice(qc * QW + ns * 512,
                                             qc * QW + (ns + 1) * 512)
                                nc.tensor.matmul(
                                    ps[:, sl_o],
                                    w2s[m][:, k * 128:(k + 1) * 128],
                                    at_tiles[k][:, sl_i],
                                    start=(k == 0), stop=(k == HPG - 1))
                        st = ost.tile([128, QW], BF16, tag="ost")
                        nc.vector.tensor_scalar_add(st[:], ps[:],
                                                    bo_sb[:, m:m + 1])
                        nc.sync.dma_start(
                            pout[m * 128:(m + 1) * 128, qc * QW:(qc + 1) * QW],
                            st[:])

                    ops_pool._qk_tag = "ops"
                    ops_pool._qk_width = QW
                    dq_pend = [None]

                    def filler_unit(kind, a, b_):
                        if kind == "qk":
                            qb = qk_proj(a, b_, ops_pool)
                            if dq_pend[0] is not None:
                                qk_rope(*dq_pend[0], pss_pool)
                            dq_pend[0] = (qb, a, b_)
                        elif kind == "rope_flush":
                            if dq_pend[0] is not None:
                                qk_rope(*dq_pend[0], pss_pool)
                                dq_pend[0] = None
                        else:
                            p3_unit(a, b_, ops_pool)

                    for qc in range(NQC):
                        if qc == 0:
                            # deferred Q projections as PE filler (every 8 steps)
                            filler = [("qk", m, ch)
                                      for ch in range(NCH - NDEF, NCH)
                                      for m in range(HPG)]
                            filler.append(("rope_flush", 0, 0))
                            fevery = 8
                        else:
                            # prior q-chunk's output projection as PE filler
                            filler = [("p3", m, qc - 1) for m in range(KT)]
                            fevery = 4
                        fi = 0
                        nstep = 0
                        for h in range(HPG):
                            dacc = dac_pool.tile([128, QW], BF16, tag="dac",
                                                 name=f"dac{qc}_{h}")
                            pso = pso_pool.tile([128, QW], F32, tag="pso",
                                                name=f"pso{qc}_{h}")
                            lagq = []
                            for step in range(ST + 2):
                                if step < ST:
                                    ms = step
                                    pss = pss_pool.tile([128, QW], F32,
                                                        tag="pss")
                                    for ns in range(2):
                                        sl = slice(ns * 512, (ns + 1) * 512)
                                        nc.tensor.matmul(
                                            pss[:, sl],
                                            qk[HPG + h][:, ms * 128:(ms + 1) * 128],
                                            qk[h][:, qc * QW + ns * 512:
                                                   qc * QW + (ns + 1) * 512],
                                            start=True, stop=True)
                                    ex = exp_pool.tile([128, QW], BF16,
                                                       tag="ex")
                                    nc.scalar.activation(
                                        ex[:], pss[:], EXP,
                                        bias=mb_sb[:, ms:ms + 1], scale=SCALE)
                                    if ms == 0:
                                        nc.vector.tensor_copy(dacc[:], ex[:])
                                    else:
                                        nc.vector.tensor_add(dacc[:], dacc[:],
                                                             ex[:])
                                if step < ST:
                                    lagq.append((ms, ex))
                                if len(lagq) > 2 or (step >= ST and lagq):
                                    pms, pex = lagq.pop(0)
                                    for ns in range(2):
                                        sl = slice(ns * 512, (ns + 1) * 512)
                                        nc.tensor.matmul(
                                            pso[:, sl],
                                            v_sb[pms][:, h * 128:(h + 1) * 128],
                                            pex[:, sl],
                                            start=(pms == 0),
                                            stop=(pms == ST - 1))
                                nstep += 1
                                if nstep % fevery == 0 and fi < len(filler):
                                    filler_unit(*filler[fi])
                                    fi += 1
                            # denominator: broadcast column-sum of dacc
                            psd = pss_pool.tile([128, QW], F32, tag="pss",
                                                name=f"psd{qc}_{h}")
                            for ns in range(2):
                                sl = slice(ns * 512, (ns + 1) * 512)
                                nc.tensor.matmul(psd[:, sl], ones_sb[:],
                                                 dacc[:, sl],
                                                 start=True, stop=True)
                            osb = osbp.tile([128, QW], F32, tag="osb")
                            nc.vector.tensor_copy(osb[:], pso[:])
                            rc = rcp.tile([128, QW], F32, tag="rc")
                            nc.vector.reciprocal_approx_fast(rc[:], psd[:])
                            nc.vector.tensor_mul(
                                at_tiles[h][:, qc * QW:(qc + 1) * QW],
                                osb[:], rc[:])
                        while fi < len(filler):
                            filler_unit(*filler[fi])
                            fi += 1

                # ---------------- Phase 3 tail: last q-chunk's projection
                with tc.tile_pool(name="ops2", bufs=3, space="PSUM") as ops2, \
                     tc.tile_pool(name="ost2", bufs=3) as ost2:
                    for m in range(KT):
                        halves = (1, 1) if m == KT - 1 else (2,)
                        done = 0
                        for nh in halves:
                            w = 512 * nh
                            ps = ops2.tile([128, QW], F32, tag="ops2")
                            for k in range(HPG):
                                for ns in range(nh):
                                    sl_o = slice(ns * 512, (ns + 1) * 512)
                                    sl_i = slice((NQC - 1) * QW + done + ns * 512,
                                                 (NQC - 1) * QW + done + (ns + 1) * 512)
                                    nc.tensor.matmul(
                                        ps[:, sl_o],
                                        w2s[m][:, k * 128:(k + 1) * 128],
                                        at_tiles[k][:, sl_i],
                                        start=(k == 0), stop=(k == HPG - 1))
                            st = ost2.tile([128, QW], BF16, tag="ost2")
                            nc.vector.tensor_scalar_add(st[:, 0:w], ps[:, 0:w],
                                                        bo_sb[:, m:m + 1])
                            nc.sync.dma_start(
                                pout[m * 128:(m + 1) * 128,
                                     (NQC - 1) * QW + done:
                                     (NQC - 1) * QW + done + w], st[:, 0:w])
                            done += w

    nc.compile()
    return nc


def _rope_tables():
    # Bug-faithful to the reference: exponent divides by EMB_DIM, not head_dim.
    angle = 1.0 / np.power(10000.0, np.arange(0, D, 2, dtype=np.float64) / E)
    t = np.arange(S, dtype=np.float64)
    freqs = np.repeat(t[:, None] * angle[None, :], 2, axis=-1)  # [S, D]
    return np.cos(freqs).astype(np.float32), np.sin(freqs).astype(np.float32)


def _prep_inputs(X, mask, W_qkv, b_qkv, W_o, b_o):
    """Build the 8 per-core input maps."""
    import ml_dtypes
    BF = ml_dtypes.bfloat16

    X = np.ascontiguousarray(np.asarray(X, dtype=np.float32))
    mask = np.asarray(mask)
    W_qkv = np.asarray(W_qkv, dtype=np.float32)
    b_qkv = np.asarray(b_qkv, dtype=np.float32)
    W_o = np.asarray(W_o, dtype=np.float32)
    b_o = np.asarray(b_o, dtype=np.float32)

    cos, sin = _rope_tables()
    cosx = np.ascontiguousarray(cos.T).astype(BF)            # [D, S]
    sinx = np.ascontiguousarray(sin.T).astype(BF)            # [D, S]
    ones = np.ones((128, 128), dtype=np.float32).astype(BF)
    # trans(q)[j] = -q[2j+1] (j<64), +q[2j-128] (j>=64), as lhsT: permT[d, j]
    permT = np.zeros((128, 128), dtype=np.float32)
    for j in range(64):
        permT[2 * j + 1, j] = -1.0
    for j in range(64, 128):
        permT[2 * (j - 64), j] = 1.0
    permT = permT.astype(BF)

    xts = [np.ascontiguousarray(X[b].T).astype(BF) for b in range(B)]
    mbs = []
    for b in range(B):
        m = np.where(mask[b] == 0, np.float32(-1e9), np.float32(0.0)).astype(np.float32)
        mbs.append(np.ascontiguousarray(m.reshape(ST, 128).T))
    bo_t = np.ascontiguousarray(b_o.reshape(KT, 128).T)
    bo_z = np.zeros_like(bo_t)

    in_maps = []
    for c in range(NCORES):
        b, g = divmod(c, NGROUP)
        qs = slice(g * GE, (g + 1) * GE)
        ks = slice(E + g * GE, E + (g + 1) * GE)
        vs = slice(2 * E + g * GE, 2 * E + (g + 1) * GE)
        w1 = np.concatenate([W_qkv[qs], W_qkv[ks]], axis=0)     # [1024, E] (Q;K)
        bqk_v = np.concatenate([b_qkv[qs], b_qkv[ks]])          # [1024]
        # pack W1^T so each m-column's 16 k-tiles are contiguous:
        # w1p[m][e_loc, k*128+col] = W1^T[k*128+e_loc, m*128+col]
        w1tt = np.ascontiguousarray(w1.T)                       # [E, 1024]
        w1pk = w1tt.reshape(KT, 128, MQK, 128).transpose(2, 1, 0, 3).reshape(
            MQK * 128, E)
        w1v_ = np.ascontiguousarray(W_qkv[vs].T)                # [E, 512] = Wv_g^T
        bv_bcast = np.broadcast_to(b_qkv[vs][None, :], (128, GE)).copy()
        w2tt = np.ascontiguousarray(W_o[:, g * GE:(g + 1) * GE].T)  # [512, E]
        w2pk = w2tt.reshape(HPG, 128, KT, 128).transpose(2, 1, 0, 3).reshape(
            KT * 128, GE)
        in_maps.append({
            "xt": xts[b],
            "w1qk": np.ascontiguousarray(w1pk).astype(BF),
            "w1v": w1v_.astype(BF),
            "w2t": np.ascontiguousarray(w2pk).astype(BF),
            "bqkv": np.ascontiguousarray(bqk_v.reshape(MQK, 128).T),
            "bvb": np.ascontiguousarray(bv_bcast),
            "bo": bo_t if g == 0 else bo_z,
            "mb": mbs[b],
            "cosx": cosx,
            "sinx": sinx,
            "ones": ones,
            "perm": permT,
        })
    return in_maps


def kernel(X, mask, W_qkv, b_qkv, W_o, b_o, _trace=False):
    from concourse.bass_utils import run_bass_kernel_spmd

    if "nc" not in _CACHE:
        _CACHE["nc"] = _build()
    nc = _CACHE["nc"]

    in_maps = _prep_inputs(X, mask, W_qkv, b_qkv, W_o, b_o)
    res = run_bass_kernel_spmd(nc, in_maps, core_ids=list(range(NCORES)),
                               trace=_trace)
    _CACHE["last_result"] = res

    out = np.empty((B, S, E), dtype=np.float32)
    for b in range(B):
        acc = res.results[b * NGROUP]["pout"].astype(np.float32)
        for g in range(1, NGROUP):
            acc += res.results[b * NGROUP + g]["pout"].astype(np.float32)
        out[b] = acc.T
    return out


# revision 17
# speedup vs baseline: 1.1943x; 1.1943x over previous
"""Trainium2 Bass kernel for a 16-head decoder self-attention block.

Reference computation (B=2, S=2048, E=2048, H=16, D=128):
    qkv = X @ W_qkv.T + b_qkv ; RoPE(Q, K) ; attn = softmax(QK^T/sqrt(D) + mask)
    out = (attn @ V reshaped) @ W_o.T + b_o

Sharding over 8 NeuronCores: data parallel over batch (2) x tensor parallel
over 4 head-groups of 4 heads each. Each core computes its group's qkv
projection, attention, and a partial (rank-512) slice of the output
projection; the host sums the 4 partials per batch element.

v4: all matmul operands bf16 (PSUM stays fp32), everything SBUF-resident.
The attention phase is ScalarE(exp)-bound with only ~80% PE duty, which
makes the PE's DVFS drop it to a lower p-state; to keep the PE saturated
(and at max clock) the kernel interleaves deferred independent matmul work
into the attention stream:
  - q-chunk 0 gets the deferred Q-projection of sequence quarters 2-3
    (only needed by q-chunk 1);
  - q-chunk 1 gets q-chunk 0's output projection (phase 3), whose PSUM
    eviction runs on DVE (tensor_scalar bias add) to keep ScalarE free;
  - the softmax denominator is accumulated on DVE in bf16 (plus one
    ones-matmul per chunk), not on the PE.
V is projected X-tile-stationary directly in [s, d] layout (no PE
transposes); Q/K evictions fuse bias (ACT) + RoPE (perm matmul + DVE).
"""

import math
import sys

import numpy as np

sys.path.insert(0, "/opt/trn_rl_repo")

B, S, E = 2, 2048, 2048
H, D = 16, 128
NCORES = 8
NGROUP = 4          # head groups (tensor parallel)
HPG = H // NGROUP   # heads per group = 4
GE = HPG * D        # group embed width = 512
KT = E // 128       # contraction tiles over E = 16
ST = S // 128       # sequence tiles = 16
MQK = 2 * HPG       # q+k m-tiles per core = 8
SCALE = 1.0 / math.sqrt(D)
NQC = 2             # q-chunks of 1024
QW = S // NQC

_CACHE = {}


def _build():
    """Build + compile the per-core Bass program (same program, all cores)."""
    import concourse.bacc as bacc
    import concourse.mybir as mybir
    import concourse.tile as tile

    F32 = mybir.dt.float32
    BF16 = mybir.dt.bfloat16
    EXP = mybir.ActivationFunctionType.Exp
    IDENT = mybir.ActivationFunctionType.Identity

    nc = bacc.Bacc("TRN2", target_bir_lowering=False, debug=False)

    xt = nc.dram_tensor("xt", [E, S], BF16, kind="ExternalInput").ap()       # X[b].T
    w1qk = nc.dram_tensor("w1qk", [MQK * 128, E], BF16, kind="ExternalInput").ap()
    w1v = nc.dram_tensor("w1v", [E, GE], BF16, kind="ExternalInput").ap()    # Wv_g^T
    w2t = nc.dram_tensor("w2t", [KT * 128, GE], BF16, kind="ExternalInput").ap()
    bqkv = nc.dram_tensor("bqkv", [128, MQK], F32, kind="ExternalInput").ap()
    bvb = nc.dram_tensor("bvb", [128, GE], F32, kind="ExternalInput").ap()   # b_v bcast
    bo = nc.dram_tensor("bo", [128, KT], F32, kind="ExternalInput").ap()
    mb = nc.dram_tensor("mb", [128, ST], F32, kind="ExternalInput").ap()     # mask bias
    cosx = nc.dram_tensor("cosx", [128, S], BF16, kind="ExternalInput").ap()
    sinx = nc.dram_tensor("sinx", [128, S], BF16, kind="ExternalInput").ap()
    ones = nc.dram_tensor("ones", [128, 128], BF16, kind="ExternalInput").ap()
    perm = nc.dram_tensor("perm", [128, 128], BF16, kind="ExternalInput").ap()
    pout = nc.dram_tensor("pout", [E, S], BF16, kind="ExternalOutput").ap()

    QC = 512
    NCH = S // QC            # 4 s-quarters
    SPC = QC // 128          # s-tiles per quarter = 4
    NDEF = 2                 # deferred Q quarters (2, 3)

    with tile.TileContext(nc) as tc:
        with tc.tile_pool(name="small", bufs=1) as spool, \
             tc.tile_pool(name="qkout", bufs=1) as qkpool, \
             tc.tile_pool(name="vout", bufs=1) as vpool, \
             tc.tile_pool(name="at", bufs=1) as atpool, \
             tc.tile_pool(name="xq", bufs=2) as xqp:
            # long-lived SBUF tensors
            qk = [qkpool.tile([128, S], BF16, tag=f"qk{m}", name=f"qk{m}")
                  for m in range(MQK)]                       # rope'd Q (0-3), K (4-7)
            v_sb = [vpool.tile([128, GE], BF16, tag=f"v{st_}", name=f"v{st_}")
                    for st_ in range(ST)]                    # V tiles [sk, d-group]
            at_tiles = [atpool.tile([128, S], BF16, tag=f"at{h}", name=f"at{h}")
                        for h in range(HPG)]

            # PE warm-up: dummy matmuls during the initial DMA window keep
            # the tensor engine's DVFS ramped before the first real chain
            with tc.tile_pool(name="warm", bufs=1) as warmp, \
                 tc.tile_pool(name="wps", bufs=2, space="PSUM") as wpsp:
                wt_ = warmp.tile([128, QC], BF16, tag="warm")
                nc.vector.memset(wt_[:], 0.0)
                for i in range(22):
                    wps = wpsp.tile([128, QC], F32, tag="wps")
                    nc.tensor.matmul(wps[:], wt_[:, 0:128], wt_[:],
                                     start=True, stop=True)

            # startup DMAs, in need order: first V chain needs xq0[k]+w1v[k]
            xq_sets = [None] * NCH
            xq_sets[0] = []
            w1v_sb = []
            for k in range(KT):
                t = xqp.tile([128, QC], BF16, tag=f"xq{k}", name=f"xq0_{k}")
                nc.sync.dma_start(t[:], xt[k * 128:(k + 1) * 128, 0:QC])
                xq_sets[0].append(t)
                w = spool.tile([128, GE], BF16, tag=f"w1v{k}", name=f"w1v{k}")
                nc.sync.dma_start(w[:], w1v[k * 128:(k + 1) * 128, :])
                w1v_sb.append(w)
            bvb_sb = spool.tile([128, GE], F32, tag="bvb")
            nc.sync.dma_start(bvb_sb[:], bvb)
            w1m = [spool.tile([128, E], BF16, tag=f"w1m{m}", name=f"w1m{m}")
                   for m in range(MQK)]
            nc.sync.dma_start(w1m[0][:], w1qk[0:128, :])
            bq_sb = spool.tile([128, MQK], F32, tag="bq")
            nc.sync.dma_start(bq_sb[:], bqkv)
            perm_sb = spool.tile([128, 128], BF16, tag="perm")
            nc.sync.dma_start(perm_sb[:], perm)
            cos_sb = spool.tile([128, S], BF16, tag="cos")
            nc.sync.dma_start(cos_sb[:], cosx)
            sin_sb = spool.tile([128, S], BF16, tag="sin")
            nc.sync.dma_start(sin_sb[:], sinx)
            for m in range(1, MQK):
                nc.sync.dma_start(w1m[m][:], w1qk[m * 128:(m + 1) * 128, :])
            ones_sb = spool.tile([128, 128], BF16, tag="ones")
            nc.sync.dma_start(ones_sb[:], ones)
            mb_sb = spool.tile([128, ST], F32, tag="mb")
            nc.sync.dma_start(mb_sb[:], mb)
            bo_sb = spool.tile([128, KT], F32, tag="bo")
            nc.sync.dma_start(bo_sb[:], bo)

            with tc.tile_pool(name="qbp", bufs=3) as qbp, \
                 tc.tile_pool(name="rap", bufs=2) as rap, \
                 tc.tile_pool(name="stp", bufs=2) as stp:

                def qk_proj(m, ch, ps_pool):
                    """Q/K m-tile projection matmuls + ACT bias evict -> qb."""
                    xq = xq_sets[ch]
                    ps_t = ps_pool.tile([128, ps_pool._qk_width], F32,
                                        tag=ps_pool._qk_tag,
                                        name=f"qps{m}_{ch}")
                    ps = ps_t[:, 0:QC]
                    for k in range(KT):
                        nc.tensor.matmul(
                            ps, w1m[m][:, k * 128:(k + 1) * 128],
                            xq[k][:], start=(k == 0), stop=(k == KT - 1))
                    qb = qbp.tile([128, QC], BF16, tag="qb")
                    nc.scalar.activation(qb[:], ps, IDENT,
                                         bias=bq_sb[:, m:m + 1], scale=1.0)
                    return qb

                def qk_rope(qb, m, ch, ps2_pool):
                    """RoPE: trans() is a signed dim-permutation matmul.
                    Emitted one unit behind the projection so the perm
                    matmul never waits on the ACT eviction."""
                    cs = slice(ch * QC, (ch + 1) * QC)
                    ps2_t = ps2_pool.tile([128, ps2_pool._qk_width], F32,
                                          tag=ps2_pool._qk_tag,
                                          name=f"qps2{m}_{ch}")
                    ps2 = ps2_t[:, 0:QC]
                    nc.tensor.matmul(ps2, perm_sb[:], qb[:],
                                     start=True, stop=True)
                    ra = rap.tile([128, QC], BF16, tag="ra")
                    nc.vector.tensor_mul(ra[:], qb[:], cos_sb[:, cs])
                    st = stp.tile([128, QC], BF16, tag="st")
                    nc.vector.tensor_mul(st[:], ps2, sin_sb[:, cs])
                    nc.vector.tensor_add(qk[m][:, cs], st[:], ra[:])

                # ---------------- Phase 1 (s-quarters; Q of quarters 2-3 deferred)
                with tc.tile_pool(name="vps", bufs=2, space="PSUM") as vps_pool, \
                     tc.tile_pool(name="qkps", bufs=2, space="PSUM") as qkps, \
                     tc.tile_pool(name="qkps2", bufs=2, space="PSUM") as qkps2:
                    qkps._qk_tag = "ps"
                    qkps._qk_width = QC
                    qkps2._qk_tag = "ps2"
                    qkps2._qk_width = QC
                    pend = [None]
                    for ch in range(NCH):
                        if ch + 1 < NCH:
                            ncs = slice((ch + 1) * QC, (ch + 2) * QC)
                            xq_sets[ch + 1] = []
                            for k in range(KT):
                                t = xqp.tile([128, QC], BF16, tag=f"xq{k}",
                                             name=f"xq{ch + 1}_{k}")
                                nc.sync.dma_start(
                                    t[:], xt[k * 128:(k + 1) * 128, ncs])
                                xq_sets[ch + 1].append(t)
                        # V projection: X-tile stationary, [s, d] output layout
                        for sl_ in range(SPC):
                            st_ = ch * SPC + sl_
                            ps = vps_pool.tile([128, GE], F32, tag="vps")
                            for k in range(KT):
                                nc.tensor.matmul(
                                    ps[:], xq_sets[ch][k][:, sl_ * 128:(sl_ + 1) * 128],
                                    w1v_sb[k][:],
                                    start=(k == 0), stop=(k == KT - 1))
                            nc.vector.tensor_add(v_sb[st_][:], ps[:], bvb_sb[:])
                        # K always; Q only for quarters 0-1 (rest deferred)
                        for m in range(MQK):
                            if m < HPG and ch >= NCH - NDEF:
                                continue
                            qb = qk_proj(m, ch, qkps)
                            if pend[0] is not None:
                                qk_rope(*pend[0], qkps2)
                            pend[0] = (qb, m, ch)
                    if pend[0] is not None:
                        qk_rope(*pend[0], qkps2)

                # W2 resident; DMAs land during early phase 2
                w2s = []
                for m in range(KT):
                    t = spool.tile([128, GE], BF16, tag=f"w2{m}", name=f"w2{m}")
                    nc.sync.dma_start(t[:], w2t[m * 128:(m + 1) * 128, :])
                    w2s.append(t)

                # ---------------- Phase 2 (+ deferred Q, + interleaved phase 3)
                with tc.tile_pool(name="ex", bufs=4) as exp_pool, \
                     tc.tile_pool(name="dac", bufs=2) as dac_pool, \
                     tc.tile_pool(name="rc", bufs=1) as rcp, \
                     tc.tile_pool(name="osb", bufs=2) as osbp, \
                     tc.tile_pool(name="ost", bufs=2) as ost, \
                     tc.tile_pool(name="pss", bufs=2, space="PSUM") as pss_pool, \
                     tc.tile_pool(name="pso", bufs=1, space="PSUM") as pso_pool, \
                     tc.tile_pool(name="ops", bufs=1, space="PSUM") as ops_pool:
                    pss_pool._qk_tag = "pss"
                    pss_pool._qk_width = QW

                    def p3_unit(m, qc, pool):
                        """One output-projection m-tile for one q-chunk."""
                        ps = pool.tile([128, QW], F32, tag="ops")
                        for k in range(HPG):
                            for ns in range(2):
                                sl_o = slice(ns * 512, (ns + 1) * 512)
                                sl_i = slice(qc * QW + ns * 512,
                                             qc * QW + (ns + 1) * 512)
                                nc.tensor.matmul(
                                    ps[:, sl_o],
                                    w2s[m][:, k * 128:(k + 1) * 128],
                                    at_tiles[k][:, sl_i],
                                    start=(k == 0), stop=(k == HPG - 1))
                        st = ost.tile([128, QW], BF16, tag="ost")
                        nc.vector.tensor_scalar_add(st[:], ps[:],
                                                    bo_sb[:, m:m + 1])
                        nc.sync.dma_start(
                            pout[m * 128:(m + 1) * 128, qc * QW:(qc + 1) * QW],
                            st[:])

                    ops_pool._qk_tag = "ops"
                    ops_pool._qk_width = QW
                    dq_pend = [None]

                    def filler_unit(kind, a, b_):
                        if kind == "qk":
                            qb = qk_proj(a, b_, ops_pool)
                            if dq_pend[0] is not None:
                                qk_rope(*dq_pend[0], pss_pool)
                            dq_pend[0] = (qb, a, b_)
                        elif kind == "rope_flush":
                            if dq_pend[0] is not None:
                                qk_rope(*dq_pend[0], pss_pool)
                                dq_pend[0] = None
                        else:
                            p3_unit(a, b_, ops_pool)

                    for qc in range(NQC):
                        if qc == 0:
                            # deferred Q projections as PE filler (every 8 steps)
                            filler = [("qk", m, ch)
                                      for ch in range(NCH - NDEF, NCH)
                                      for m in range(HPG)]
                            filler.append(("rope_flush", 0, 0))
                            fevery = 8
                        else:
                            # prior q-chunk's output projection as PE filler
                            filler = [("p3", m, qc - 1) for m in range(KT)]
                            fevery = 4
                        fi = 0
                        nstep = 0
                        for h in range(HPG):
                            dacc = dac_pool.tile([128, QW], BF16, tag="dac",
                                                 name=f"dac{qc}_{h}")
                            pso = pso_pool.tile([128, QW], F32, tag="pso",
                                                name=f"pso{qc}_{h}")
                            lagq = []
                            for step in range(ST + 2):
                                if step < ST:
                                    ms = step
                                    pss = pss_pool.tile([128, QW], F32,
                                                        tag="pss")
                                    for ns in range(2):
                                        sl = slice(ns * 512, (ns + 1) * 512)
                                        nc.tensor.matmul(
                                            pss[:, sl],
                                            qk[HPG + h][:, ms * 128:(ms + 1) * 128],
                                            qk[h][:, qc * QW + ns * 512:
                                                   qc * QW + (ns + 1) * 512],
                                            start=True, stop=True)
                                    ex = exp_pool.tile([128, QW], BF16,
                                                       tag="ex")
                                    nc.scalar.activation(
                                        ex[:], pss[:], EXP,
                                        bias=mb_sb[:, ms:ms + 1], scale=SCALE)
                                    if ms == 0:
                                        nc.vector.tensor_copy(dacc[:], ex[:])
                                    else:
                                        nc.vector.tensor_add(dacc[:], dacc[:],
                                                             ex[:])
                                if step < ST:
                                    lagq.append((ms, ex))
                                if len(lagq) > 2 or (step >= ST and lagq):
                                    pms, pex = lagq.pop(0)
                                    for ns in range(2):
                                        sl = slice(ns * 512, (ns + 1) * 512)
                                        nc.tensor.matmul(
                                            pso[:, sl],
                                            v_sb[pms][:, h * 128:(h + 1) * 128],
                                            pex[:, sl],
                                            start=(pms == 0),
                                            stop=(pms == ST - 1))
                                nstep += 1
                                if nstep % fevery == 0 and fi < len(filler):
                                    filler_unit(*filler[fi])
                                    fi += 1
                            # denominator: broadcast column-sum of dacc
                            psd = pss_pool.tile([128, QW], F32, tag="pss",
                                                name=f"psd{qc}_{h}")
                            for ns in range(2):
                                sl = slice(ns * 512, (ns + 1) * 512)
                                nc.tensor.matmul(psd[:, sl], ones_sb[:],
                                                 dacc[:, sl],
                                                 start=True, stop=True)
                            osb = osbp.tile([128, QW], F32, tag="osb")
                            nc.vector.tensor_copy(osb[:], pso[:])
                            rc = rcp.tile([128, QW], F32, tag="rc")
                            nc.vector.reciprocal_approx_fast(rc[:], psd[:])
                            nc.vector.tensor_mul(
                                at_tiles[h][:, qc * QW:(qc + 1) * QW],
                                osb[:], rc[:])
                        while fi < len(filler):
                            filler_unit(*filler[fi])
                            fi += 1

                # ---------------- Phase 3 tail: last q-chunk's projection
                with tc.tile_pool(name="ops2", bufs=3, space="PSUM") as ops2, \
                     tc.tile_pool(name="ost2", bufs=3) as ost2:
                    for m in range(KT):
                        halves = (1, 1) if m == KT - 1 else (2,)
                        done = 0
                        for nh in halves:
                            w = 512 * nh
                            ps = ops2.tile([128, QW], F32, tag="ops2")
                            for k in range(HPG):
                                for ns in range(nh):
                                    sl_o = slice(ns * 512, (ns + 1) * 512)
                                    sl_i = slice((NQC - 1) * QW + done + ns * 512,
                                                 (NQC - 1) * QW + done + (ns + 1) * 512)
                                    nc.tensor.matmul(
                                        ps[:, sl_o],
                                        w2s[m][:, k * 128:(k + 1) * 128],
                                        at_tiles[k][:, sl_i],
                                        start=(k == 0), stop=(k == HPG - 1))
                            st = ost2.tile([128, QW], BF16, tag="ost2")
                            nc.vector.tensor_scalar_add(st[:, 0:w], ps[:, 0:w],
                                                        bo_sb[:, m:m + 1])
                            nc.sync.dma_start(
                                pout[m * 128:(m + 1) * 128,
                                     (NQC - 1) * QW + done:
                                     (NQC - 1) * QW + done + w], st[:, 0:w])
                            done += w

    nc.compile()
    return nc


def _rope_tables():
    # Bug-faithful to the reference: exponent divides by EMB_DIM, not head_dim.
    angle = 1.0 / np.power(10000.0, np.arange(0, D, 2, dtype=np.float64) / E)
    t = np.arange(S, dtype=np.float64)
    freqs = np.repeat(t[:, None] * angle[None, :], 2, axis=-1)  # [S, D]
    return np.cos(freqs).astype(np.float32), np.sin(freqs).astype(np.float32)


def _prep_inputs(X, mask, W_qkv, b_qkv, W_o, b_o):
    """Build the 8 per-core input maps."""
    import ml_dtypes
    BF = ml_dtypes.bfloat16

    X = np.ascontiguousarray(np.asarray(X, dtype=np.float32))
    mask = np.asarray(mask)
    W_qkv = np.asarray(W_qkv, dtype=np.float32)
    b_qkv = np.asarray(b_qkv, dtype=np.float32)
    W_o = np.asarray(W_o, dtype=np.float32)
    b_o = np.asarray(b_o, dtype=np.float32)

    cos, sin = _rope_tables()
    cosx = np.ascontiguousarray(cos.T).astype(BF)            # [D, S]
    sinx = np.ascontiguousarray(sin.T).astype(BF)            # [D, S]
    ones = np.ones((128, 128), dtype=np.float32).astype(BF)
    # trans(q)[j] = -q[2j+1] (j<64), +q[2j-128] (j>=64), as lhsT: permT[d, j]
    permT = np.zeros((128, 128), dtype=np.float32)
    for j in range(64):
        permT[2 * j + 1, j] = -1.0
    for j in range(64, 128):
        permT[2 * (j - 64), j] = 1.0
    permT = permT.astype(BF)

    xts = [np.ascontiguousarray(X[b].T).astype(BF) for b in range(B)]
    mbs = []
    for b in range(B):
        m = np.where(mask[b] == 0, np.float32(-1e9), np.float32(0.0)).astype(np.float32)
        mbs.append(np.ascontiguousarray(m.reshape(ST, 128).T))
    bo_t = np.ascontiguousarray(b_o.reshape(KT, 128).T)
    bo_z = np.zeros_like(bo_t)

    in_maps = []
    for c in range(NCORES):
        b, g = divmod(c, NGROUP)
        qs = slice(g * GE, (g + 1) * GE)
        ks = slice(E + g * GE, E + (g + 1) * GE)
        vs = slice(2 * E + g * GE, 2 * E + (g + 1) * GE)
        w1 = np.concatenate([W_qkv[qs], W_qkv[ks]], axis=0)     # [1024, E] (Q;K)
        bqk_v = np.concatenate([b_qkv[qs], b_qkv[ks]])          # [1024]
        # pack W1^T so each m-column's 16 k-tiles are contiguous:
        # w1p[m][e_loc, k*128+col] = W1^T[k*128+e_loc, m*128+col]
        w1tt = np.ascontiguousarray(w1.T)                       # [E, 1024]
        w1pk = w1tt.reshape(KT, 128, MQK, 128).transpose(2, 1, 0, 3).reshape(
            MQK * 128, E)
        w1v_ = np.ascontiguousarray(W_qkv[vs].T)                # [E, 512] = Wv_g^T
        bv_bcast = np.broadcast_to(b_qkv[vs][None, :], (128, GE)).copy()
        w2tt = np.ascontiguousarray(W_o[:, g * GE:(g + 1) * GE].T)  # [512, E]
        w2pk = w2tt.reshape(HPG, 128, KT, 128).transpose(2, 1, 0, 3).reshape(
            KT * 128, GE)
        in_maps.append({
            "xt": xts[b],
            "w1qk": np.ascontiguousarray(w1pk).astype(BF),
            "w1v": w1v_.astype(BF),
            "w2t": np.ascontiguousarray(w2pk).astype(BF),
            "bqkv": np.ascontiguousarray(bqk_v.reshape(MQK, 128).T),
            "bvb": np.ascontiguousarray(bv_bcast),
            "bo": bo_t if g == 0 else bo_z,
            "mb": mbs[b],
            "cosx": cosx,
            "sinx": sinx,
            "ones": ones,
            "perm": permT,
        })
    return in_maps


def kernel(X, mask, W_qkv, b_qkv, W_o, b_o, _trace=False):
    from concourse.bass_utils import run_bass_kernel_spmd

    if "nc" not in _CACHE:
        _CACHE["nc"] = _build()
    nc = _CACHE["nc"]

    in_maps = _prep_inputs(X, mask, W_qkv, b_qkv, W_o, b_o)
    res = run_bass_kernel_spmd(nc, in_maps, core_ids=list(range(NCORES)),
                               trace=_trace)
    _CACHE["last_result"] = res

    out = np.empty((B, S, E), dtype=np.float32)
    for b in range(B):
        acc = res.results[b * NGROUP]["pout"].astype(np.float32)
        for g in range(1, NGROUP):
            acc += res.results[b * NGROUP + g]["pout"].astype(np.float32)
        out[b] = acc.T
    return out
